# revision 36
# baseline (speedup 1.0000x reference)
"""Trainium2 Bass kernel for CAConv2 (coordinate-attention + 3x3 conv block).

Shapes (hardcoded): x (8, 128, 128, 128) f32; data-parallel over batch,
one image per NeuronCore (8 cores).

Schedule notes (from trace analysis):
- chunk-completion DMA semaphores lag the data by 2-5us (engine-queue
  skew), so the attention front-end is built around few, coarse chunks
  and work that tolerates late sems;
- the a_w chain is the conv-start critical path: single-row ps_xw
  matmuls avoid the parity-sum, BN reads PSUM directly;
- rows 96-127 trees + block-B a_h are deferred behind early gating with
  explicit ordering edges (the Tile scheduler otherwise hoists them in
  front of the a_w chain, stalling the DVE on a late chunk sem).
"""

import numpy as np
import ml_dtypes

import concourse.bacc as bacc
import concourse.tile as tile
from concourse import mybir
from concourse.bass import ds
from concourse.bass import _add_dep_helper as add_dep
from concourse.bass_utils import run_bass_kernel_spmd

BF16 = mybir.dt.bfloat16
F32 = mybir.dt.float32
C, H, W, MIP = 128, 128, 128, 8
WP = W + 4  # padded width: cols [2, 130) hold data, 0/1 and 130/131 are zero
HP = H + 2  # padded height: rows [1, 129) hold data
EPS = 1e-5
AF = mybir.ActivationFunctionType
ALU = mybir.AluOpType

_CACHE = {}


def build_nc():
    nc = bacc.Bacc(num_swdge_queues=2)
    xp = nc.declare_dram_parameter("x", [C, H * W], BF16, isOutput=False)
    w1t = nc.declare_dram_parameter("w1t", [C, MIP], BF16, isOutput=False)
    w1ts = nc.declare_dram_parameter("w1ts", [C, 3 * MIP], BF16, isOutput=False)
    wht = nc.declare_dram_parameter("wht", [MIP, C], BF16, isOutput=False)
    wwt = nc.declare_dram_parameter("wwt", [MIP, C], BF16, isOutput=False)
    # wct[i, k, o] = wc[o, i, k//3, k%3]
    wct = nc.declare_dram_parameter("wct", [C, 9 * C], BF16, isOutput=False)
    # p8 cols: 0: s1/6, 1: t1f/6, 2: s1, 3: t1f+3   (t1f = s1*b1 + be1 - m1*s1)
    p8 = nc.declare_dram_parameter("p8", [MIP, 4], F32, isOutput=False)
    # p128 cols: 0: bh, 1: bw, 2: s2, 3: b2 (= bc*s2 + be2 - m2*s2)
    p128 = nc.declare_dram_parameter("p128", [C, 4], F32, isOutput=False)
    outp = nc.declare_dram_parameter("out", [C, H, W], BF16, isOutput=True)

    c1, c2, c3 = 7.0 / 128, 3.0 / 128, 1.0 / 128

    with tile.TileContext(nc) as tc:
        with (
            tc.tile_pool(name="sing", bufs=1) as sing,
            tc.tile_pool(name="pp", bufs=2) as pp,
            tc.tile_pool(name="small", bufs=1) as small,
        ):
            xs = sing.tile([C, H * W], BF16)
            ug = sing.tile([C, HP, WP], BF16)
            s32 = sing.tile([C, H, 4], F32)

            # rows 0-63 (+ small weights) ride the sync ring (descriptors go
            # out the moment the preamble barrier drops); rows 64-127 ride
            # the gpsimd SWDGE ring concurrently — two rings halve per-chunk
            # sem lag. gpsimd runs no compute during the input window.
            nc.sync.dma_start(out=xs[:, ds(0, 8 * W)], in_=xp[:, ds(0, 8 * W)])
            w1ts_sb = sing.tile([C, 3, MIP], BF16)
            nc.sync.dma_start(
                out=w1ts_sb, in_=w1ts.rearrange("c (r m) -> c r m", r=3)
            )
            for r0, nr in [(8, 24), (32, 32), (64, 32), (96, 16), (112, 8), (120, 8)]:
                nc.gpsimd.dma_start(
                    out=xs[:, ds(r0 * W, nr * W)],
                    in_=xp[:, ds(r0 * W, nr * W)],
                )
            w1t_sb = sing.tile([C, MIP], BF16)
            nc.sync.dma_start(out=w1t_sb, in_=w1t[:, :])
            wht_sb = sing.tile([MIP, C], BF16)
            nc.sync.dma_start(out=wht_sb, in_=wht[:, :])
            wwt_sb = sing.tile([MIP, C], BF16)
            nc.sync.dma_start(out=wwt_sb, in_=wwt[:, :])
            p8_sb = sing.tile([MIP, 4], F32)
            nc.sync.dma_start(out=p8_sb, in_=p8[:, :])
            p128_sb = sing.tile([C, 4], F32)
            nc.sync.dma_start(out=p128_sb, in_=p128[:, :])
            # bulky conv weights: floor their modeled time past the input
            # window so their transfer doesn't contend with x (needed ~27us)
            wct_sb = sing.tile([C, 9, C], BF16)
            nc.sync.dma_start(
                out=wct_sb, in_=wct.rearrange("i (k o) -> i k o", k=9)
            )

            # conv padding border of ug (DVE is idle this early)
            nc.vector.memset(ug[:, 0, :], 0.0)
            nc.vector.memset(ug[:, HP - 1, :], 0.0)
            nc.vector.memset(ug[:, 1 : HP - 1, 0:2], 0.0)
            nc.vector.memset(ug[:, 1 : HP - 1, WP - 2 : WP], 0.0)

            # preload ACT function tables off the critical path
            dummy = small.tile([C, 2], F32)
            nc.vector.memset(dummy, 0.0)
            dump = small.tile([C, 2], F32)
            for fn in (AF.Silu, AF.Sigmoid, AF.Relu):
                nc.scalar.activation(dump, dummy, fn, bias=0.0, scale=1.0)

            with tc.tile_pool(name="psA", bufs=1, space="PSUM") as psA:
                # x_w: single-row matmuls with range-prescaled w1 accumulate
                # the weighted row-pool directly onto ONE (8, W) psum tile
                ps_xw = psA.tile([MIP, W], F32, tag="xw")
                ps_yh = psA.tile([MIP, H], F32, tag="yh")
                ps_ah = psA.tile([C, H], F32, tag="ah")
                ah_sb = small.tile([C, H], BF16)

                def row_mms(r0, nr):
                    for row in range(r0, r0 + nr):
                        nc.tensor.matmul(
                            ps_xw,
                            w1ts_sb[:, min(row // 32, 2), :],
                            xs[:, ds(row * W, W)],
                            start=(row == 0),
                            stop=(row == 127),
                        )

                def emit_tree(r0, nr):
                    # 32-col segment sums for rows [r0, r0+nr), nr <= 32.
                    # Returns (first, last) instructions for ordering edges.
                    eng = nc.vector
                    xc = xs[:, ds(r0 * W, nr * W)].rearrange(
                        "p (y q s) -> p y q s", q=4, s=32
                    )
                    t1 = pp.tile([C, 32, 4, 16], BF16, tag="t1")
                    i0 = eng.tensor_add(
                        t1[:, :nr], xc[:, :, :, 0:16], xc[:, :, :, 16:32]
                    )
                    t2 = pp.tile([C, 32, 4, 8], BF16, tag="t2")
                    eng.tensor_add(t2[:, :nr], t1[:, :nr, :, 0:8], t1[:, :nr, :, 8:16])
                    t3 = pp.tile([C, 32, 4, 4], BF16, tag="t3")
                    eng.tensor_add(t3[:, :nr], t2[:, :nr, :, 0:4], t2[:, :nr, :, 4:8])
                    t4 = pp.tile([C, 32, 4, 2], BF16, tag="t4")
                    eng.tensor_add(t4[:, :nr], t3[:, :nr, :, 0:2], t3[:, :nr, :, 2:4])
                    sl = s32[:, ds(r0, nr), :]
                    i1 = eng.tensor_add(sl, t4[:, :nr, :, 0], t4[:, :nr, :, 1])
                    return i0, i1

                def xh_pool(rlo, rhi, tg):
                    # combine s32 rows [rlo, rhi) into the pooled xh (bf16)
                    n = rhi - rlo
                    slh = s32[:, ds(rlo, n), :]
                    tmpA = pp.tile([C, n], F32, tag=tg + "tmpA")
                    nc.vector.tensor_add(tmpA, slh[:, :, 2], slh[:, :, 3])
                    m0 = pp.tile([C, n], F32, tag=tg + "m0")
                    nc.vector.tensor_scalar_mul(m0, slh[:, :, 0], c1)
                    m1 = pp.tile([C, n], F32, tag=tg + "m1")
                    nc.vector.scalar_tensor_tensor(
                        out=m1, in0=slh[:, :, 1], scalar=c2, in1=m0,
                        op0=ALU.mult, op1=ALU.add,
                    )
                    xhp = pp.tile([C, n], BF16, tag=tg + "xhp")
                    ilast = nc.vector.scalar_tensor_tensor(
                        out=xhp, in0=tmpA, scalar=c3, in1=m1,
                        op0=ALU.mult, op1=ALU.add,
                    )
                    return xhp, ilast

                def bn_hswish(src, dst, n, tg):
                    # dst = h_swish(s1*src + t1f) for an (MIP, n) slice (DVE)
                    z6 = pp.tile([MIP, n], F32, tag=tg + "bn_z6")
                    i0 = nc.vector.tensor_scalar(
                        out=z6, in0=src, scalar1=p8_sb[:, 0:1],
                        scalar2=p8_sb[:, 1:2], op0=ALU.mult, op1=ALU.add,
                    )
                    r = pp.tile([MIP, n], F32, tag=tg + "bn_r")
                    nc.vector.tensor_scalar(
                        out=r, in0=z6, scalar1=6.0, scalar2=3.0,
                        op0=ALU.mult, op1=ALU.add,
                    )
                    rc = pp.tile([MIP, n], F32, tag=tg + "bn_rc")
                    nc.vector.tensor_scalar(
                        out=rc, in0=r, scalar1=0.0, scalar2=6.0,
                        op0=ALU.max, op1=ALU.min,
                    )
                    i1 = nc.vector.tensor_mul(dst, z6, rc)
                    return i0, i1

                def gate_rows(rlo, rhi):
                    first = last = None
                    for y in range(rlo, rhi):
                        last = nc.vector.scalar_tensor_tensor(
                            out=ug[:, 1 + y, 2 : 2 + W],
                            in0=xs[:, ds(y * W, W)],
                            scalar=ah_sb[:, y : y + 1],
                            in1=aw_sb,
                            op0=ALU.mult,
                            op1=ALU.mult,
                        )
                        if first is None:
                            first = last
                    return first, last

                # ---- chunk-chasing: row matmuls + trees + block A ----
                row_mms(0, 8)
                row_mms(8, 24)
                emit_tree(0, 32)
                row_mms(32, 32)
                emit_tree(32, 64 - 32)
                # block A: rows 0-63 -> a_h
                xhpA, _ = xh_pool(0, 64, "va")
                nc.tensor.matmul(ps_yh[:, 0:64], w1t_sb, xhpA, start=True, stop=True)
                xh_shA = pp.tile([MIP, 64], BF16, tag="xh_shA")
                bnA = bn_hswish(ps_yh[:, 0:64], xh_shA, 64, "va")
                nc.tensor.matmul(ps_ah[:, 0:64], wht_sb, xh_shA, start=True, stop=True)
                nc.scalar.activation(
                    ah_sb[:, 0:64], ps_ah[:, 0:64],
                    AF.Sigmoid, bias=p128_sb[:, 0:1], scale=1.0,
                )
                row_mms(64, 32)
                tree64 = emit_tree(64, 32)
                # keep tree(64,96) out of block A's DVE stream
                add_dep(tree64[0].ins, bnA[1].ins, sync=False, reason="order: t64 after bnA")
                row_mms(96, 16)
                row_mms(112, 16)

                # ---- a_w chain: the conv-start critical path ----
                # h_swish as z6 * min(relu(z+3), 6): the relu runs on Scalar
                # in parallel with the DVE z6, leaving one DVE op in series
                z6w = small.tile([MIP, W], F32)
                i_z6w = nc.vector.tensor_scalar(
                    out=z6w, in0=ps_xw, scalar1=p8_sb[:, 0:1],
                    scalar2=p8_sb[:, 1:2], op0=ALU.mult, op1=ALU.add,
                )
                uw = small.tile([MIP, W], F32)
                nc.scalar.activation(
                    uw, ps_xw, AF.Relu,
                    bias=p8_sb[:, 3:4], scale=p8_sb[:, 2:3],
                )
                xw_s = small.tile([MIP, W], BF16)
                i_xws = nc.vector.scalar_tensor_tensor(
                    out=xw_s, in0=uw, scalar=6.0, in1=z6w,
                    op0=ALU.min, op1=ALU.mult,
                )
                bnW = (i_z6w, i_xws)
                add_dep(bnW[0].ins, tree64[1].ins, sync=False, reason="order: aw after t64")

                # warm-keeper matmuls fill the PE-idle handoff window so HAM
                # doesn't re-throttle right before the conv: batch 0 scribbles
                # on ps_aw (the real a_w matmul start=True clears it), batch 1
                # on ps_xw (its readers z6w/uw are done by then)
                ps_aw = psA.tile([C, W], F32, tag="aw")
                wfirst = None
                for _ in range(10):
                    wi = nc.tensor.matmul(
                        ps_aw[0:MIP, :], w1ts_sb[:, 0, :], xs[:, 0:W],
                        start=True, stop=True,
                    )
                    if wfirst is None:
                        wfirst = wi
                add_dep(wfirst.ins, i_z6w.ins, sync=False,
                        reason="order: warm0 after z6w")
                i_awmm = nc.tensor.matmul(ps_aw, wwt_sb, xw_s, start=True, stop=True)
                w1first = None
                for _ in range(25):
                    wi = nc.tensor.matmul(
                        ps_xw, w1ts_sb[:, 0, :], xs[:, 0:W],
                        start=True, stop=True,
                    )
                    if w1first is None:
                        w1first = wi
                add_dep(w1first.ins, i_awmm.ins, sync=False,
                        reason="order: warm1 after awMM")
                aw_sb = small.tile([C, W], BF16)
                nc.scalar.activation(
                    aw_sb, ps_aw, AF.Sigmoid, bias=p128_sb[:, 1:2], scale=1.0
                )

                g0 = gate_rows(0, 32)
                add_dep(g0[0].ins, bnW[1].ins, sync=False, reason="order: gate0 after aw bn")

                # ---- 3x3 conv + BN2 + SiLU ----
                with (
                    tc.tile_pool(name="psB", bufs=4, space="PSUM") as psB,
                    tc.tile_pool(name="obp", bufs=4) as obp,
                ):
                    def conv_block(rb):
                        pso = psB.tile([C, 4, W], F32, tag="pso")
                        for k in range(9):
                            dy, dx = k // 3, k % 3
                            nc.tensor.matmul(
                                pso,
                                wct_sb[:, k, :],
                                ug[:, 4 * rb + dy : 4 * rb + dy + 4,
                                   1 + dx : 1 + dx + W],
                                start=(k == 0),
                                stop=(k == 8),
                            )
                        ob = obp.tile([C, 4, W], BF16, tag="ob")
                        nc.scalar.activation(
                            ob, pso, AF.Silu,
                            bias=p128_sb[:, 3:4], scale=p128_sb[:, 2:3],
                        )
                        nc.sync.dma_start(
                            out=outp[:, 4 * rb : 4 * rb + 4, :], in_=ob
                        )

                    # conv block rb reads gated rows 4rb-1 .. 4rb+4: every
                    # gate_rows() below precedes the conv blocks it covers.
                    for rb in range(7):          # rows <= 28
                        conv_block(rb)

                    # deferred: rows 96-127 tree + block-B a_h, interleaved
                    # with gating so ah_B is ready well before conv block 15
                    tree96 = emit_tree(96, 32)
                    add_dep(tree96[0].ins, g0[1].ins, sync=False,
                            reason="order: t96 after gate0")
                    xhpB, xhpB_i = xh_pool(64, 128, "vb")
                    nc.tensor.matmul(
                        ps_yh[:, 64:128], w1t_sb, xhpB, start=True, stop=True
                    )

                    g1a = gate_rows(32, 48)
                    add_dep(g1a[0].ins, xhpB_i.ins, sync=False,
                            reason="order: gate1a after xhpB")

                    for rb in range(7, 11):      # rows <= 44
                        conv_block(rb)

                    xh_shB = pp.tile([MIP, 64], BF16, tag="xh_shB")
                    bnB = bn_hswish(ps_yh[:, 64:128], xh_shB, 64, "vb")
                    add_dep(bnB[0].ins, g1a[1].ins, sync=False,
                            reason="order: bnB after gate1a")
                    g1b = gate_rows(48, 64)
                    add_dep(g1b[0].ins, bnB[1].ins, sync=False,
                            reason="order: gate1b after bnB")

                    nc.tensor.matmul(
                        ps_ah[:, 64:128], wht_sb, xh_shB, start=True, stop=True
                    )
                    nc.scalar.activation(
                        ah_sb[:, 64:128], ps_ah[:, 64:128],
                        AF.Sigmoid, bias=p128_sb[:, 0:1], scale=1.0,
                    )

                    for rb in range(11, 15):     # rows <= 60
                        conv_block(rb)

                    g2 = gate_rows(64, 128)
                    add_dep(g2[0].ins, g1b[1].ins, sync=False,
                            reason="order: gate2 after gate1b")

                    for rb in range(15, H // 4 - 1):
                        conv_block(rb)

                    # tail: rows 124-125 as a 4-row-read block, rows 126-127
                    # as a final 2-row block so the last evacuation is short
                    for rlo, nr in ((124, 2), (126, 2)):
                        pso = psB.tile([C, 4, W], F32, tag="pso")
                        for k in range(9):
                            dy, dx = k // 3, k % 3
                            nc.tensor.matmul(
                                pso[:, 0:nr, :],
                                wct_sb[:, k, :],
                                ug[:, rlo + dy : rlo + dy + nr,
                                   1 + dx : 1 + dx + W],
                                start=(k == 0),
                                stop=(k == 8),
                            )
                        obh = obp.tile([C, 4, W], BF16, tag="ob")
                        nc.scalar.activation(
                            obh[:, 0:nr, :], pso[:, 0:nr, :], AF.Silu,
                            bias=p128_sb[:, 3:4], scale=p128_sb[:, 2:3],
                        )
                        nc.sync.dma_start(
                            out=outp[:, rlo : rlo + nr, :], in_=obh[:, 0:nr, :]
                        )

    nc.compile()
    return nc


def prep_inputs(x, w1, b1, g1, be1, m1, v1, wh, bh, ww, bw, wc, bc, g2, be2, m2, v2):
    """Host-side prep: per-core input maps (weights replicated)."""
    bf = ml_dtypes.bfloat16
    N = x.shape[0]
    s1 = (g1 / np.sqrt(v1 + EPS)).astype(np.float64)
    t1f = s1 * b1 + be1 - m1 * s1
    p8 = np.stack([s1 / 6.0, t1f / 6.0, s1, t1f + 3.0], axis=1).astype(np.float32)
    s2 = (g2 / np.sqrt(v2 + EPS)).astype(np.float64)
    b2 = bc * s2 + be2 - m2 * s2
    p128 = np.stack([bh, bw, s2, b2], axis=1).astype(np.float32)
    cc = np.array([7.0 / 128, 3.0 / 128, 1.0 / 128])
    w1ts = np.stack([w1.T * c for c in cc], axis=1)              # (C, 3, MIP)
    shared = {
        "w1t": np.ascontiguousarray(w1.T).astype(bf),            # (C, MIP)
        "w1ts": np.ascontiguousarray(w1ts.reshape(C, 3 * MIP)).astype(bf),
        "wht": np.ascontiguousarray(wh.T).astype(bf),            # (MIP, C)
        "wwt": np.ascontiguousarray(ww.T).astype(bf),            # (MIP, C)
        "wct": np.ascontiguousarray(
            np.transpose(wc, (1, 2, 3, 0)).reshape(C, 9 * C)
        ).astype(bf),                                            # [i, (ky kx), o]
        "p8": p8,
        "p128": p128,
    }
    in_maps = []
    for n in range(N):
        m = dict(shared)
        m["x"] = np.ascontiguousarray(x[n].reshape(C, H * W)).astype(bf)
        in_maps.append(m)
    return in_maps


def run(inputs, trace=False):
    if "nc" not in _CACHE:
        _CACHE["nc"] = build_nc()
    nc = _CACHE["nc"]
    in_maps = prep_inputs(**inputs)
    res = run_bass_kernel_spmd(nc, in_maps, core_ids=list(range(8)), trace=trace)
    out = np.stack([np.asarray(res.results[i]["out"]) for i in range(8)], axis=0)
    return out.astype(np.float32), res


def kernel(**inputs) -> np.ndarray:
    out, _ = run(inputs, trace=False)
    return out


# revision 37
# speedup vs baseline: 1.0168x; 1.0168x over previous
"""Trainium2 Bass kernel for CAConv2 (coordinate-attention + 3x3 conv block).

Shapes (hardcoded): x (8, 128, 128, 128) f32; data-parallel over batch,
one image per NeuronCore (8 cores).

Schedule notes (from trace analysis):
- chunk-completion DMA semaphores lag the data by 2-5us (engine-queue
  skew), so the attention front-end is built around few, coarse chunks
  and work that tolerates late sems;
- the a_w chain is the conv-start critical path: single-row ps_xw
  matmuls avoid the parity-sum, BN reads PSUM directly;
- rows 96-127 trees + block-B a_h are deferred behind early gating with
  explicit ordering edges (the Tile scheduler otherwise hoists them in
  front of the a_w chain, stalling the DVE on a late chunk sem).
"""

import numpy as np
import ml_dtypes

import concourse.bacc as bacc
import concourse.tile as tile
from concourse import mybir
from concourse.bass import ds
from concourse.bass import _add_dep_helper as add_dep
from concourse.bass_utils import run_bass_kernel_spmd

BF16 = mybir.dt.bfloat16
F32 = mybir.dt.float32
C, H, W, MIP = 128, 128, 128, 8
WP = W + 4  # padded width: cols [2, 130) hold data, 0/1 and 130/131 are zero
HP = H + 2  # padded height: rows [1, 129) hold data
EPS = 1e-5
AF = mybir.ActivationFunctionType
ALU = mybir.AluOpType

_CACHE = {}


def build_nc():
    nc = bacc.Bacc(num_swdge_queues=2)
    xp = nc.declare_dram_parameter("x", [C, H * W], BF16, isOutput=False)
    w1t = nc.declare_dram_parameter("w1t", [C, MIP], BF16, isOutput=False)
    w1ts = nc.declare_dram_parameter("w1ts", [C, 3 * MIP], BF16, isOutput=False)
    wht = nc.declare_dram_parameter("wht", [MIP, C], BF16, isOutput=False)
    wwt = nc.declare_dram_parameter("wwt", [MIP, C], BF16, isOutput=False)
    # wct[i, k, o] = wc[o, i, k//3, k%3]
    wct = nc.declare_dram_parameter("wct", [C, 9 * C], BF16, isOutput=False)
    # p8 cols: 0: s1/6, 1: t1f/6, 2: s1, 3: t1f+3   (t1f = s1*b1 + be1 - m1*s1)
    p8 = nc.declare_dram_parameter("p8", [MIP, 4], F32, isOutput=False)
    # p128 cols: 0: bh, 1: bw, 2: s2, 3: b2 (= bc*s2 + be2 - m2*s2)
    p128 = nc.declare_dram_parameter("p128", [C, 4], F32, isOutput=False)
    outp = nc.declare_dram_parameter("out", [C, H, W], BF16, isOutput=True)

    c1, c2, c3 = 7.0 / 128, 3.0 / 128, 1.0 / 128

    with tile.TileContext(nc) as tc:
        with (
            tc.tile_pool(name="sing", bufs=1) as sing,
            tc.tile_pool(name="pp", bufs=2) as pp,
            tc.tile_pool(name="small", bufs=1) as small,
        ):
            xs = sing.tile([C, H * W], BF16)
            ug = sing.tile([C, HP, WP], BF16)
            s32 = sing.tile([C, H, 4], F32)

            # rows 0-63 (+ small weights) ride the sync ring (descriptors go
            # out the moment the preamble barrier drops); rows 64-127 ride
            # the gpsimd SWDGE ring concurrently — two rings halve per-chunk
            # sem lag. gpsimd runs no compute during the input window.
            nc.sync.dma_start(out=xs[:, ds(0, 8 * W)], in_=xp[:, ds(0, 8 * W)])
            w1ts_sb = sing.tile([C, 3, MIP], BF16)
            nc.sync.dma_start(
                out=w1ts_sb, in_=w1ts.rearrange("c (r m) -> c r m", r=3)
            )
            for r0, nr in [(8, 24), (32, 32), (64, 32), (96, 16), (112, 8), (120, 8)]:
                nc.gpsimd.dma_start(
                    out=xs[:, ds(r0 * W, nr * W)],
                    in_=xp[:, ds(r0 * W, nr * W)],
                )
            w1t_sb = sing.tile([C, MIP], BF16)
            nc.sync.dma_start(out=w1t_sb, in_=w1t[:, :])
            wht_sb = sing.tile([MIP, C], BF16)
            nc.sync.dma_start(out=wht_sb, in_=wht[:, :])
            wwt_sb = sing.tile([MIP, C], BF16)
            nc.sync.dma_start(out=wwt_sb, in_=wwt[:, :])
            p8_sb = sing.tile([MIP, 4], F32)
            nc.sync.dma_start(out=p8_sb, in_=p8[:, :])
            p128_sb = sing.tile([C, 4], F32)
            nc.sync.dma_start(out=p128_sb, in_=p128[:, :])
            # bulky conv weights: floor their modeled time past the input
            # window so their transfer doesn't contend with x (needed ~27us)
            wct_sb = sing.tile([C, 9, C], BF16)
            with tc.tile_wait_until(0.021):
                nc.sync.dma_start(
                    out=wct_sb, in_=wct.rearrange("i (k o) -> i k o", k=9)
                )

            # conv padding border of ug (DVE is idle this early)
            nc.vector.memset(ug[:, 0, :], 0.0)
            nc.vector.memset(ug[:, HP - 1, :], 0.0)
            nc.vector.memset(ug[:, 1 : HP - 1, 0:2], 0.0)
            nc.vector.memset(ug[:, 1 : HP - 1, WP - 2 : WP], 0.0)

            # preload ACT function tables off the critical path
            dummy = small.tile([C, 2], F32)
            nc.vector.memset(dummy, 0.0)
            dump = small.tile([C, 2], F32)
            for fn in (AF.Silu, AF.Sigmoid, AF.Relu):
                nc.scalar.activation(dump, dummy, fn, bias=0.0, scale=1.0)

            with tc.tile_pool(name="psA", bufs=1, space="PSUM") as psA:
                # x_w: single-row matmuls with range-prescaled w1 accumulate
                # the weighted row-pool directly onto ONE (8, W) psum tile
                ps_xw = psA.tile([MIP, W], F32, tag="xw")
                ps_yh = psA.tile([MIP, H], F32, tag="yh")
                ps_ah = psA.tile([C, H], F32, tag="ah")
                ah_sb = small.tile([C, H], BF16)

                def row_mms(r0, nr):
                    for row in range(r0, r0 + nr):
                        nc.tensor.matmul(
                            ps_xw,
                            w1ts_sb[:, min(row // 32, 2), :],
                            xs[:, ds(row * W, W)],
                            start=(row == 0),
                            stop=(row == 127),
                        )

                def emit_tree(r0, nr):
                    # 32-col segment sums for rows [r0, r0+nr), nr <= 32.
                    # Returns (first, last) instructions for ordering edges.
                    eng = nc.vector
                    xc = xs[:, ds(r0 * W, nr * W)].rearrange(
                        "p (y q s) -> p y q s", q=4, s=32
                    )
                    t1 = pp.tile([C, 32, 4, 16], BF16, tag="t1")
                    i0 = eng.tensor_add(
                        t1[:, :nr], xc[:, :, :, 0:16], xc[:, :, :, 16:32]
                    )
                    t2 = pp.tile([C, 32, 4, 8], BF16, tag="t2")
                    eng.tensor_add(t2[:, :nr], t1[:, :nr, :, 0:8], t1[:, :nr, :, 8:16])
                    t3 = pp.tile([C, 32, 4, 4], BF16, tag="t3")
                    eng.tensor_add(t3[:, :nr], t2[:, :nr, :, 0:4], t2[:, :nr, :, 4:8])
                    t4 = pp.tile([C, 32, 4, 2], BF16, tag="t4")
                    eng.tensor_add(t4[:, :nr], t3[:, :nr, :, 0:2], t3[:, :nr, :, 2:4])
                    sl = s32[:, ds(r0, nr), :]
                    i1 = eng.tensor_add(sl, t4[:, :nr, :, 0], t4[:, :nr, :, 1])
                    return i0, i1

                def xh_pool(rlo, rhi, tg):
                    # combine s32 rows [rlo, rhi) into the pooled xh (bf16)
                    n = rhi - rlo
                    slh = s32[:, ds(rlo, n), :]
                    tmpA = pp.tile([C, n], F32, tag=tg + "tmpA")
                    nc.vector.tensor_add(tmpA, slh[:, :, 2], slh[:, :, 3])
                    m0 = pp.tile([C, n], F32, tag=tg + "m0")
                    nc.vector.tensor_scalar_mul(m0, slh[:, :, 0], c1)
                    m1 = pp.tile([C, n], F32, tag=tg + "m1")
                    nc.vector.scalar_tensor_tensor(
                        out=m1, in0=slh[:, :, 1], scalar=c2, in1=m0,
                        op0=ALU.mult, op1=ALU.add,
                    )
                    xhp = pp.tile([C, n], BF16, tag=tg + "xhp")
                    ilast = nc.vector.scalar_tensor_tensor(
                        out=xhp, in0=tmpA, scalar=c3, in1=m1,
                        op0=ALU.mult, op1=ALU.add,
                    )
                    return xhp, ilast

                def bn_hswish(src, dst, n, tg):
                    # dst = h_swish(s1*src + t1f) for an (MIP, n) slice (DVE)
                    z6 = pp.tile([MIP, n], F32, tag=tg + "bn_z6")
                    i0 = nc.vector.tensor_scalar(
                        out=z6, in0=src, scalar1=p8_sb[:, 0:1],
                        scalar2=p8_sb[:, 1:2], op0=ALU.mult, op1=ALU.add,
                    )
                    r = pp.tile([MIP, n], F32, tag=tg + "bn_r")
                    nc.vector.tensor_scalar(
                        out=r, in0=z6, scalar1=6.0, scalar2=3.0,
                        op0=ALU.mult, op1=ALU.add,
                    )
                    rc = pp.tile([MIP, n], F32, tag=tg + "bn_rc")
                    nc.vector.tensor_scalar(
                        out=rc, in0=r, scalar1=0.0, scalar2=6.0,
                        op0=ALU.max, op1=ALU.min,
                    )
                    i1 = nc.vector.tensor_mul(dst, z6, rc)
                    return i0, i1

                def gate_rows(rlo, rhi):
                    first = last = None
                    for y in range(rlo, rhi):
                        last = nc.vector.scalar_tensor_tensor(
                            out=ug[:, 1 + y, 2 : 2 + W],
                            in0=xs[:, ds(y * W, W)],
                            scalar=ah_sb[:, y : y + 1],
                            in1=aw_sb,
                            op0=ALU.mult,
                            op1=ALU.mult,
                        )
                        if first is None:
                            first = last
                    return first, last

                # ---- chunk-chasing: row matmuls + trees + block A ----
                row_mms(0, 8)
                row_mms(8, 24)
                emit_tree(0, 32)
                row_mms(32, 32)
                emit_tree(32, 64 - 32)
                # block A: rows 0-63 -> a_h
                xhpA, _ = xh_pool(0, 64, "va")
                nc.tensor.matmul(ps_yh[:, 0:64], w1t_sb, xhpA, start=True, stop=True)
                xh_shA = pp.tile([MIP, 64], BF16, tag="xh_shA")
                bnA = bn_hswish(ps_yh[:, 0:64], xh_shA, 64, "va")
                nc.tensor.matmul(ps_ah[:, 0:64], wht_sb, xh_shA, start=True, stop=True)
                nc.scalar.activation(
                    ah_sb[:, 0:64], ps_ah[:, 0:64],
                    AF.Sigmoid, bias=p128_sb[:, 0:1], scale=1.0,
                )
                row_mms(64, 32)
                tree64 = emit_tree(64, 32)
                # keep tree(64,96) out of block A's DVE stream
                add_dep(tree64[0].ins, bnA[1].ins, sync=False, reason="order: t64 after bnA")
                row_mms(96, 16)
                row_mms(112, 16)

                # ---- a_w chain: the conv-start critical path ----
                # h_swish as z6 * min(relu(z+3), 6): the relu runs on Scalar
                # in parallel with the DVE z6, leaving one DVE op in series
                z6w = small.tile([MIP, W], F32)
                i_z6w = nc.vector.tensor_scalar(
                    out=z6w, in0=ps_xw, scalar1=p8_sb[:, 0:1],
                    scalar2=p8_sb[:, 1:2], op0=ALU.mult, op1=ALU.add,
                )
                uw = small.tile([MIP, W], F32)
                nc.scalar.activation(
                    uw, ps_xw, AF.Relu,
                    bias=p8_sb[:, 3:4], scale=p8_sb[:, 2:3],
                )
                xw_s = small.tile([MIP, W], BF16)
                i_xws = nc.vector.scalar_tensor_tensor(
                    out=xw_s, in0=uw, scalar=6.0, in1=z6w,
                    op0=ALU.min, op1=ALU.mult,
                )
                bnW = (i_z6w, i_xws)
                add_dep(bnW[0].ins, tree64[1].ins, sync=False, reason="order: aw after t64")

                # warm-keeper matmuls fill the PE-idle handoff window so HAM
                # doesn't re-throttle right before the conv: batch 0 scribbles
                # on ps_aw (the real a_w matmul start=True clears it), batch 1
                # on ps_xw (its readers z6w/uw are done by then)
                ps_aw = psA.tile([C, W], F32, tag="aw")
                wfirst = None
                for _ in range(10):
                    wi = nc.tensor.matmul(
                        ps_aw[0:MIP, :], w1ts_sb[:, 0, :], xs[:, 0:W],
                        start=True, stop=True,
                    )
                    if wfirst is None:
                        wfirst = wi
                add_dep(wfirst.ins, i_z6w.ins, sync=False,
                        reason="order: warm0 after z6w")
                i_awmm = nc.tensor.matmul(ps_aw, wwt_sb, xw_s, start=True, stop=True)
                w1first = None
                for _ in range(25):
                    wi = nc.tensor.matmul(
                        ps_xw, w1ts_sb[:, 0, :], xs[:, 0:W],
                        start=True, stop=True,
                    )
                    if w1first is None:
                        w1first = wi
                add_dep(w1first.ins, i_awmm.ins, sync=False,
                        reason="order: warm1 after awMM")
                aw_sb = small.tile([C, W], BF16)
                nc.scalar.activation(
                    aw_sb, ps_aw, AF.Sigmoid, bias=p128_sb[:, 1:2], scale=1.0
                )

                g0 = gate_rows(0, 32)
                add_dep(g0[0].ins, bnW[1].ins, sync=False, reason="order: gate0 after aw bn")

                # ---- 3x3 conv + BN2 + SiLU ----
                with (
                    tc.tile_pool(name="psB", bufs=4, space="PSUM") as psB,
                    tc.tile_pool(name="obp", bufs=4) as obp,
                ):
                    def conv_block(rb):
                        pso = psB.tile([C, 4, W], F32, tag="pso")
                        for k in range(9):
                            dy, dx = k // 3, k % 3
                            nc.tensor.matmul(
                                pso,
                                wct_sb[:, k, :],
                                ug[:, 4 * rb + dy : 4 * rb + dy + 4,
                                   1 + dx : 1 + dx + W],
                                start=(k == 0),
                                stop=(k == 8),
                            )
                        ob = obp.tile([C, 4, W], BF16, tag="ob")
                        nc.scalar.activation(
                            ob, pso, AF.Silu,
                            bias=p128_sb[:, 3:4], scale=p128_sb[:, 2:3],
                        )
                        nc.sync.dma_start(
                            out=outp[:, 4 * rb : 4 * rb + 4, :], in_=ob
                        )

                    # conv block rb reads gated rows 4rb-1 .. 4rb+4: every
                    # gate_rows() below precedes the conv blocks it covers.
                    for rb in range(7):          # rows <= 28
                        conv_block(rb)

                    # deferred: rows 96-127 tree + block-B a_h, interleaved
                    # with gating so ah_B is ready well before conv block 15
                    tree96 = emit_tree(96, 32)
                    add_dep(tree96[0].ins, g0[1].ins, sync=False,
                            reason="order: t96 after gate0")
                    xhpB, xhpB_i = xh_pool(64, 128, "vb")
                    nc.tensor.matmul(
                        ps_yh[:, 64:128], w1t_sb, xhpB, start=True, stop=True
                    )

                    g1a = gate_rows(32, 48)
                    add_dep(g1a[0].ins, xhpB_i.ins, sync=False,
                            reason="order: gate1a after xhpB")

                    for rb in range(7, 11):      # rows <= 44
                        conv_block(rb)

                    xh_shB = pp.tile([MIP, 64], BF16, tag="xh_shB")
                    bnB = bn_hswish(ps_yh[:, 64:128], xh_shB, 64, "vb")
                    add_dep(bnB[0].ins, g1a[1].ins, sync=False,
                            reason="order: bnB after gate1a")
                    g1b = gate_rows(48, 64)
                    add_dep(g1b[0].ins, bnB[1].ins, sync=False,
                            reason="order: gate1b after bnB")

                    nc.tensor.matmul(
                        ps_ah[:, 64:128], wht_sb, xh_shB, start=True, stop=True
                    )
                    nc.scalar.activation(
                        ah_sb[:, 64:128], ps_ah[:, 64:128],
                        AF.Sigmoid, bias=p128_sb[:, 0:1], scale=1.0,
                    )

                    for rb in range(11, 15):     # rows <= 60
                        conv_block(rb)

                    g2 = gate_rows(64, 128)
                    add_dep(g2[0].ins, g1b[1].ins, sync=False,
                            reason="order: gate2 after gate1b")

                    for rb in range(15, H // 4 - 1):
                        conv_block(rb)

                    # tail: rows 124-125 as a 4-row-read block, rows 126-127
                    # as a final 2-row block so the last evacuation is short
                    for rlo, nr in ((124, 2), (126, 2)):
                        pso = psB.tile([C, 4, W], F32, tag="pso")
                        for k in range(9):
                            dy, dx = k // 3, k % 3
                            nc.tensor.matmul(
                                pso[:, 0:nr, :],
                                wct_sb[:, k, :],
                                ug[:, rlo + dy : rlo + dy + nr,
                                   1 + dx : 1 + dx + W],
                                start=(k == 0),
                                stop=(k == 8),
                            )
                        obh = obp.tile([C, 4, W], BF16, tag="ob")
                        nc.scalar.activation(
                            obh[:, 0:nr, :], pso[:, 0:nr, :], AF.Silu,
                            bias=p128_sb[:, 3:4], scale=p128_sb[:, 2:3],
                        )
                        nc.sync.dma_start(
                            out=outp[:, rlo : rlo + nr, :], in_=obh[:, 0:nr, :]
                        )

    nc.compile()
    return nc


def prep_inputs(x, w1, b1, g1, be1, m1, v1, wh, bh, ww, bw, wc, bc, g2, be2, m2, v2):
    """Host-side prep: per-core input maps (weights replicated)."""
    bf = ml_dtypes.bfloat16
    N = x.shape[0]
    s1 = (g1 / np.sqrt(v1 + EPS)).astype(np.float64)
    t1f = s1 * b1 + be1 - m1 * s1
    p8 = np.stack([s1 / 6.0, t1f / 6.0, s1, t1f + 3.0], axis=1).astype(np.float32)
    s2 = (g2 / np.sqrt(v2 + EPS)).astype(np.float64)
    b2 = bc * s2 + be2 - m2 * s2
    p128 = np.stack([bh, bw, s2, b2], axis=1).astype(np.float32)
    cc = np.array([7.0 / 128, 3.0 / 128, 1.0 / 128])
    w1ts = np.stack([w1.T * c for c in cc], axis=1)              # (C, 3, MIP)
    shared = {
        "w1t": np.ascontiguousarray(w1.T).astype(bf),            # (C, MIP)
        "w1ts": np.ascontiguousarray(w1ts.reshape(C, 3 * MIP)).astype(bf),
        "wht": np.ascontiguousarray(wh.T).astype(bf),            # (MIP, C)
        "wwt": np.ascontiguousarray(ww.T).astype(bf),            # (MIP, C)
        "wct": np.ascontiguousarray(
            np.transpose(wc, (1, 2, 3, 0)).reshape(C, 9 * C)
        ).astype(bf),                                            # [i, (ky kx), o]
        "p8": p8,
        "p128": p128,
    }
    in_maps = []
    for n in range(N):
        m = dict(shared)
        m["x"] = np.ascontiguousarray(x[n].reshape(C, H * W)).astype(bf)
        in_maps.append(m)
    return in_maps


def run(inputs, trace=False):
    if "nc" not in _CACHE:
        _CACHE["nc"] = build_nc()
    nc = _CACHE["nc"]
    in_maps = prep_inputs(**inputs)
    res = run_bass_kernel_spmd(nc, in_maps, core_ids=list(range(8)), trace=trace)
    out = np.stack([np.asarray(res.results[i]["out"]) for i in range(8)], axis=0)
    return out.astype(np.float32), res


def kernel(**inputs) -> np.ndarray:
    out, _ = run(inputs, trace=False)
    return out
